# revision 2
# baseline (speedup 1.0000x reference)
"""Multi-head attention (QKV projection + masked softmax + PV) on 8 TRN2
NeuronCores.

Sharding: data-parallel over batch (B=2 -> 2 groups of 4 cores), tensor
parallel over heads (16 heads -> 4 heads per core). Each core computes full
F x T attention for its 4 heads.

Per-core device algorithm (everything kept transposed so the softmax
reduction lands on the TensorE contraction dim):
  Q^T[h,f] = wq^T @ from^T     (fp32r matmuls, tf32-level precision)
  K^T[h,t] = wk^T @ to^T
  V[t,hh]  = to^T^T @ wv  (+ ones column per head for the softmax sums)
  S^T[t,f] = K^T(stationary) x Q^T(moving)  -> PSUM
  E = exp(S^T/8) (ScalarE, PSUM->SBUF, bf16) ; E *= mask^T (bf16)
  ctx^T[h,f] (+ sums[f] via the ones column) = sum_t V x E
  out = ctx^T * (1/sums)  broadcast via a K=1 matmul
Host does the cheap layout work: pre-transposes from/to/mask, slices
weights per head group, and transposes the [4,64,2048] per-core results
back into [B,F,N,H].
"""

import os
import sys

for _p in ("/opt/trn_rl_repo",):
    if os.path.isdir(_p) and _p not in sys.path:
        sys.path.insert(0, _p)

import numpy as np
import ml_dtypes

import concourse.tile as tile
from concourse import bacc, mybir
from concourse.bass_utils import run_bass_kernel_spmd

B, F, T, D, N, H = 2, 2048, 2048, 1024, 16, 64
NCORES = 8
HPC = N // (NCORES // B)  # heads per core = 4
NG = HPC // 2             # groups of 2 heads (128 partitions) = 2
FB = 512                  # f-block (psum bank width in fp32)
NJ = F // FB              # 4
NT = T // 128             # 16 t-tiles
NK = D // 128             # 8 contraction tiles

F32 = mybir.dt.float32
F32R = mybir.dt.float32r
BF16 = mybir.dt.bfloat16


def _program():
    nc = bacc.Bacc(None, target_bir_lowering=False)
    fromT = nc.declare_dram_parameter("fromT", [D, F], F32R, isOutput=False)
    toT = nc.declare_dram_parameter("toT", [D, T], F32R, isOutput=False)
    maskT = nc.declare_dram_parameter("maskT", [T, F], BF16, isOutput=False)
    wq = nc.declare_dram_parameter("wq", [D, HPC * H], F32R, isOutput=False)
    wk = nc.declare_dram_parameter("wk", [D, HPC * H], F32R, isOutput=False)
    wv = nc.declare_dram_parameter("wv", [D, HPC * H], F32R, isOutput=False)
    bqk = nc.declare_dram_parameter("bqk", [128, 2 * NG], F32, isOutput=False)
    bv = nc.declare_dram_parameter("bv", [1, HPC * H], F32R, isOutput=False)
    ones = nc.declare_dram_parameter("ones", [1, 128], F32R, isOutput=False)
    out_ctx = nc.declare_dram_parameter("out_ctx", [HPC, H, F], F32, isOutput=True)

    with tile.TileContext(nc) as tc:
        with tc.tile_pool(name="persist", bufs=1) as persist:
            QT = persist.tile([128, NG, F], F32R)       # [h-in-group, g, f]
            KT = persist.tile([128, NG, T], F32R)
            Vsb = persist.tile([128, NT, HPC * (H + 1)], BF16)  # [t%128, ti, (n,h|1)]
            bias_sb = persist.tile([128, 2 * NG], F32)
            bv_sb = persist.tile([1, HPC * H], F32R)
            ones_sb = persist.tile([1, 128], F32R)
            nc.sync.dma_start(bias_sb[:], bqk[:])
            nc.sync.dma_start(bv_sb[:], bv[:])
            nc.sync.dma_start(ones_sb[:], ones[:])
            for nl in range(HPC):
                nc.vector.memset(Vsb[:, :, nl * (H + 1) + H], 1.0)

            # ---- phase 1: projections ----
            with (
                tc.tile_pool(name="p1", bufs=1) as p1,
                tc.tile_pool(name="ps1", bufs=3, space="PSUM") as ps1,
            ):
                toT_sb = p1.tile([128, NK, T], F32R)
                fromT_sb = p1.tile([128, NK, F], F32R)
                wq_sb = p1.tile([128, NK, HPC * H], F32R)
                wk_sb = p1.tile([128, NK, HPC * H], F32R)
                wv_sb = p1.tile([128, NK, HPC * H], F32R)
                nc.sync.dma_start(wq_sb[:], wq[:].rearrange("(k p) m -> p k m", p=128))
                nc.sync.dma_start(wk_sb[:], wk[:].rearrange("(k p) m -> p k m", p=128))
                nc.sync.dma_start(wv_sb[:], wv[:].rearrange("(k p) m -> p k m", p=128))
                for k in range(NK):
                    nc.sync.dma_start(toT_sb[:, k, :], toT[k * 128:(k + 1) * 128, :])
                    nc.sync.dma_start(fromT_sb[:, k, :], fromT[k * 128:(k + 1) * 128, :])

                for w_sb, src_sb, dst, bcol in (
                    (wq_sb, fromT_sb, QT, 0),
                    (wk_sb, toT_sb, KT, NG),
                ):
                    for g in range(NG):
                        for j in range(NJ):
                            ps_qk = ps1.tile([128, FB], F32, tag="qk")
                            for k in range(NK):
                                nc.tensor.matmul(
                                    ps_qk[:],
                                    w_sb[:, k, g * 128:(g + 1) * 128],
                                    src_sb[:, k, j * FB:(j + 1) * FB],
                                    start=(k == 0),
                                    stop=(k == NK - 1),
                                )
                            nc.vector.tensor_scalar_add(
                                dst[:, g, j * FB:(j + 1) * FB],
                                ps_qk[:],
                                bias_sb[:, bcol + g:bcol + g + 1],
                            )

                for ti in range(NT):
                    ps_v = ps1.tile([128, HPC * H], F32, tag="v")
                    for k in range(NK):
                        nc.tensor.matmul(
                            ps_v[:],
                            toT_sb[:, k, ti * 128:(ti + 1) * 128],
                            wv_sb[:, k, :],
                            start=(k == 0),
                            stop=False,
                        )
                    nc.tensor.matmul(
                        ps_v[:], ones_sb[0:1, 0:128], bv_sb[0:1, :],
                        start=False, stop=True,
                    )
                    for nl in range(HPC):
                        nc.vector.tensor_copy(
                            Vsb[:, ti, nl * (H + 1):nl * (H + 1) + H],
                            ps_v[:, nl * H:(nl + 1) * H],
                        )

            # ---- phase 2: attention ----
            with (
                tc.tile_pool(name="p2", bufs=2) as p2,
                tc.tile_pool(name="p2s", bufs=3) as p2s,
                tc.tile_pool(name="ps_s", bufs=2, space="PSUM") as ps_s,
                tc.tile_pool(name="ps_c", bufs=2, space="PSUM") as ps_c,
                tc.tile_pool(name="ps_b", bufs=1, space="PSUM") as ps_b,
            ):
                maskT_re = maskT[:].rearrange("(a p) f -> p a f", p=128)
                for j in range(NJ):
                    mask_j = p2.tile([128, NT, FB], BF16, tag="mask")
                    nc.sync.dma_start(mask_j[:], maskT_re[:, :, j * FB:(j + 1) * FB])
                    for n in range(HPC):
                        g, half = divmod(n, 2)
                        hp = half * 64
                        expS = p2.tile([128, NT, FB], BF16, tag="expS")
                        ps_ctx = ps_c.tile([H + 1, FB], F32)
                        for q in range(NT // 2):
                            ps_sq = ps_s.tile([128, 2, FB], F32)
                            for i in range(2):
                                ti = q * 2 + i
                                nc.tensor.matmul(
                                    ps_sq[:, i, :],
                                    KT[hp:hp + 64, g, ti * 128:(ti + 1) * 128],
                                    QT[hp:hp + 64, g, j * FB:(j + 1) * FB],
                                    start=True, stop=True,
                                )
                            nc.scalar.activation(
                                expS[:, 2 * q:2 * q + 2, :],
                                ps_sq[:],
                                mybir.ActivationFunctionType.Exp,
                                scale=0.125,
                            )
                            nc.vector.tensor_mul(
                                expS[:, 2 * q:2 * q + 2, :],
                                expS[:, 2 * q:2 * q + 2, :],
                                mask_j[:, 2 * q:2 * q + 2, :],
                            )
                            for i in range(2):
                                ti = q * 2 + i
                                nc.tensor.matmul(
                                    ps_ctx[:],
                                    Vsb[:, ti, n * (H + 1):(n + 1) * (H + 1)],
                                    expS[:, ti, :],
                                    start=(ti == 0),
                                    stop=(ti == NT - 1),
                                )
                        ctx_sb = p2s.tile([H + 1, FB], F32, tag="ctx")
                        nc.vector.tensor_copy(ctx_sb[:], ps_ctx[:])
                        recip = p2s.tile([1, FB], F32, tag="recip")
                        nc.vector.reciprocal(recip[:], ctx_sb[H:H + 1, :])
                        recip_r = p2s.tile([1, FB], F32R, tag="recipr")
                        nc.vector.tensor_copy(recip_r[:], recip[:])
                        ps_bc = ps_b.tile([H, FB], F32)
                        nc.tensor.matmul(
                            ps_bc[:], ones_sb[0:1, 0:H], recip_r[0:1, :],
                            start=True, stop=True,
                        )
                        out_sb = p2s.tile([H, FB], F32, tag="out")
                        nc.vector.tensor_mul(out_sb[:], ctx_sb[0:H, :], ps_bc[:])
                        nc.sync.dma_start(
                            out_ctx[n, :, j * FB:(j + 1) * FB], out_sb[:]
                        )

    nc.compile()
    return nc


_compiled = None


def _get_compiled():
    global _compiled
    if _compiled is None:
        _compiled = _program()
    return _compiled


def _round_tf32(x):
    """Round fp32 -> tf32 (RNE on the low 13 mantissa bits), keep fp32 bits."""
    v = np.ascontiguousarray(x, dtype=np.float32).view(np.uint32)
    r = (v + np.uint32(0xFFF) + ((v >> np.uint32(13)) & np.uint32(1))) & np.uint32(
        0xFFFFE000
    )
    return r.view(np.float32)


def make_in_maps(from_tensor, to_tensor, attention_mask, wq, bq, wk, bk, wv, bv):
    from_tensor = np.asarray(from_tensor, dtype=np.float32)
    to_tensor = np.asarray(to_tensor, dtype=np.float32)
    attention_mask = np.asarray(attention_mask)
    wq = np.asarray(wq, dtype=np.float32)
    wk = np.asarray(wk, dtype=np.float32)
    wv = np.asarray(wv, dtype=np.float32)
    bq = np.asarray(bq, dtype=np.float32)
    bk = np.asarray(bk, dtype=np.float32)
    bv = np.asarray(bv, dtype=np.float32)

    fromT_b = [_round_tf32(from_tensor[b].T) for b in range(B)]
    toT_b = [_round_tf32(to_tensor[b].T) for b in range(B)]
    maskT_b = [
        attention_mask[b].T.astype(ml_dtypes.bfloat16) for b in range(B)
    ]
    ones_arr = np.ones((1, 128), dtype=np.float32)

    in_maps = []
    for c in range(NCORES):
        b, hb = divmod(c, NCORES // B)
        hs = hb * HPC
        bq_dev = bq[hs:hs + HPC].reshape(NG, 128).T
        bk_dev = bk[hs:hs + HPC].reshape(NG, 128).T
        in_maps.append(
            dict(
                fromT=fromT_b[b],
                toT=toT_b[b],
                maskT=maskT_b[b],
                wq=_round_tf32(wq[:, hs:hs + HPC, :].reshape(D, HPC * H)),
                wk=_round_tf32(wk[:, hs:hs + HPC, :].reshape(D, HPC * H)),
                wv=_round_tf32(wv[:, hs:hs + HPC, :].reshape(D, HPC * H)),
                bqk=np.ascontiguousarray(
                    np.concatenate([bq_dev, bk_dev], axis=1), dtype=np.float32
                ),
                bv=_round_tf32(bv[hs:hs + HPC].reshape(1, HPC * H)),
                ones=ones_arr,
            )
        )
    return in_maps


def gather_output(results):
    out = np.empty((B, F, N, H), dtype=np.float32)
    for c in range(NCORES):
        b, hb = divmod(c, NCORES // B)
        hs = hb * HPC
        ctx = results[c]["out_ctx"]  # [HPC, H, F]
        out[b, :, hs:hs + HPC, :] = ctx.transpose(2, 0, 1)
    return out


def run_sharded(inputs, **run_kwargs):
    """Run the SPMD kernel; returns (output, BassKernelResults)."""
    nc = _get_compiled()
    in_maps = make_in_maps(**inputs)
    res = run_bass_kernel_spmd(nc, in_maps, list(range(NCORES)), **run_kwargs)
    return gather_output(res.results), res


def kernel(**inputs):
    out, _ = run_sharded(inputs)
    return out


# revision 7
# speedup vs baseline: 1.1137x; 1.1137x over previous
"""Multi-head attention (QKV projection + masked softmax + PV) on 8 TRN2
NeuronCores.

Sharding: data-parallel over batch (B=2 -> 2 groups of 4 cores), tensor
parallel over heads (16 heads -> 4 heads per core). Each core computes full
F x T attention for its 4 heads.

Per-core device algorithm (kept transposed so the softmax reduction lands on
the TensorE contraction dim; all matmuls bf16, PSUM fp32):
  Q^T[h,f] = wq^T @ from^T        K^T[h,t] = wk^T @ to^T
  V[t,hh]  = to^T^T @ wv   (+ a ones column per head for the softmax sums)
  S^T[t,f] = K^T(stationary) x Q^T(moving)   [head pairs row-tiled: the two
             heads of a 128-partition group occupy PE row groups 0-1 / 2-3
             and run concurrently]
  E = exp(S^T/8) (ScalarE, PSUM->SBUF, bf16);  E *= mask^T (bf16)
  ctx^T[h,f] (+ sums[f] via the ones column) = sum_t V x E
  out = ctx^T * (1/sums)   reciprocals batched 4-per-j on strided partitions,
                           broadcast down the 64 h-partitions via K=1 matmul
Host does the cheap layout work: pre-transposes from/to/mask (bf16), slices
weights per head group, transposes the [4,64,2048] per-core results back into
[B,F,N,H].
"""

import os
import sys

for _p in ("/opt/trn_rl_repo",):
    if os.path.isdir(_p) and _p not in sys.path:
        sys.path.insert(0, _p)

import numpy as np
import ml_dtypes

import concourse.tile as tile
from concourse import bacc, mybir
from concourse.bass_utils import run_bass_kernel_spmd

B, F, T, D, N, H = 2, 2048, 2048, 1024, 16, 64
NCORES = 8
HPC = N // (NCORES // B)  # heads per core = 4
NG = HPC // 2             # 128-partition head groups (2 heads each) = 2
FB = 512                  # f-block (psum bank width in fp32)
NJ = F // FB              # 4
NT = T // 128             # 16 t-tiles
NK = D // 128             # 8 contraction tiles
HP1 = H + 1               # head V columns incl. the ones column

F32 = mybir.dt.float32
F32R = mybir.dt.float32r
BF16 = mybir.dt.bfloat16


def _program():
    nc = bacc.Bacc(None, target_bir_lowering=False)
    fromT = nc.declare_dram_parameter("fromT", [D, F], BF16, isOutput=False)
    toT = nc.declare_dram_parameter("toT", [D, T], BF16, isOutput=False)
    maskT = nc.declare_dram_parameter("maskT", [T, F], BF16, isOutput=False)
    wq = nc.declare_dram_parameter("wq", [D, HPC * H], BF16, isOutput=False)
    wk = nc.declare_dram_parameter("wk", [D, HPC * H], BF16, isOutput=False)
    wv = nc.declare_dram_parameter("wv", [D, HPC * H], BF16, isOutput=False)
    bqk = nc.declare_dram_parameter("bqk", [128, 2 * NG], F32, isOutput=False)
    bv = nc.declare_dram_parameter("bv", [1, HPC * H], BF16, isOutput=False)
    ones_bf = nc.declare_dram_parameter("ones_bf", [1, 128], BF16, isOutput=False)
    ones_r = nc.declare_dram_parameter("ones_r", [128, H], F32R, isOutput=False)
    out_ctx = nc.declare_dram_parameter("out_ctx", [HPC, H, F], F32, isOutput=True)

    with tile.TileContext(nc) as tc:
        with tc.tile_pool(name="persist", bufs=1) as persist:
            QT = persist.tile([128, NG, F], BF16)       # [h-in-group, g, f]
            KT = persist.tile([128, NG, T], BF16)
            Vsb = persist.tile([128, NT, HPC * HP1], BF16)
            bias_sb = persist.tile([128, 2 * NG], F32)
            bv_sb = persist.tile([1, HPC * H], BF16)
            ones_bf_sb = persist.tile([1, 128], BF16)
            ones_r_sb = persist.tile([128, H], F32R)
            nc.sync.dma_start(bias_sb[:], bqk[:])
            nc.sync.dma_start(bv_sb[:], bv[:])
            nc.sync.dma_start(ones_bf_sb[:], ones_bf[:])
            nc.sync.dma_start(ones_r_sb[:], ones_r[:])
            for nl in range(HPC):
                nc.vector.memset(Vsb[:, :, nl * HP1 + H], 1.0)

            # ---- phase 1: projections ----
            with (
                tc.tile_pool(name="p1", bufs=1) as p1,
                tc.tile_pool(name="ps1", bufs=3, space="PSUM") as ps1,
            ):
                toT_sb = p1.tile([128, NK, T], BF16)
                fromT_sb = p1.tile([128, NK, F], BF16)
                wq_sb = p1.tile([128, NK, HPC * H], BF16)
                wk_sb = p1.tile([128, NK, HPC * H], BF16)
                wv_sb = p1.tile([128, NK, HPC * H], BF16)
                nc.sync.dma_start(wq_sb[:], wq[:].rearrange("(k p) m -> p k m", p=128))
                nc.sync.dma_start(wk_sb[:], wk[:].rearrange("(k p) m -> p k m", p=128))
                nc.sync.dma_start(wv_sb[:], wv[:].rearrange("(k p) m -> p k m", p=128))
                for k in range(NK):
                    nc.sync.dma_start(toT_sb[:, k, :], toT[k * 128:(k + 1) * 128, :])
                    nc.sync.dma_start(fromT_sb[:, k, :], fromT[k * 128:(k + 1) * 128, :])

                for w_sb, src_sb, dst, bcol in (
                    (wk_sb, toT_sb, KT, NG),
                    (wq_sb, fromT_sb, QT, 0),
                ):
                    for g in range(NG):
                        for j in range(NJ):
                            ps_qk = ps1.tile([128, FB], F32, tag="qk")
                            for k in range(NK):
                                nc.tensor.matmul(
                                    ps_qk[:],
                                    w_sb[:, k, g * 128:(g + 1) * 128],
                                    src_sb[:, k, j * FB:(j + 1) * FB],
                                    start=(k == 0),
                                    stop=(k == NK - 1),
                                )
                            nc.vector.tensor_scalar_add(
                                dst[:, g, j * FB:(j + 1) * FB],
                                ps_qk[:],
                                bias_sb[:, bcol + g:bcol + g + 1],
                            )

                for ti in range(NT):
                    ps_v = ps1.tile([128, HPC * H], F32, tag="v")
                    for k in range(NK):
                        nc.tensor.matmul(
                            ps_v[:],
                            toT_sb[:, k, ti * 128:(ti + 1) * 128],
                            wv_sb[:, k, :],
                            start=(k == 0),
                            stop=False,
                        )
                    nc.tensor.matmul(
                        ps_v[:], ones_bf_sb[0:1, 0:128], bv_sb[0:1, :],
                        start=False, stop=True,
                    )
                    for nl in range(HPC):
                        nc.vector.tensor_copy(
                            Vsb[:, ti, nl * HP1:nl * HP1 + H],
                            ps_v[:, nl * H:(nl + 1) * H],
                        )

            # ---- phase 2: attention ----
            with (
                tc.tile_pool(name="p2", bufs=2) as p2,
                tc.tile_pool(name="p2e", bufs=3) as p2e,
                tc.tile_pool(name="p2s", bufs=3) as p2s,
                tc.tile_pool(name="p2r", bufs=2) as p2r,
                tc.tile_pool(name="ps_s", bufs=1, space="PSUM") as ps_s,
                tc.tile_pool(name="ps_c", bufs=1, space="PSUM") as ps_c,
            ):
                maskT_re = maskT[:].rearrange("(a p) f -> p a f", p=128)
                for j in range(NJ):
                    mask_j = p2.tile([128, NT, FB], BF16, tag="mask")
                    nc.sync.dma_start(mask_j[:], maskT_re[:, :, j * FB:(j + 1) * FB])
                    sums_g = p2r.tile([128, FB], F32, tag="sums")
                    ctx_keep = []
                    for gp in range(NG):
                        nA, nB = 2 * gp, 2 * gp + 1
                        ps_ctx = {}
                        exq = {}
                        ps_ctx[0] = ps_c.tile([HP1, FB], F32, tag="ctxA", name="ctxA")
                        ps_ctx[1] = ps_c.tile([HP1, FB], F32, tag="ctxB", name="ctxB")
                        for q in range(NT // 2):
                            ps_sq = {
                                0: ps_s.tile([128, 2, FB], F32, tag="sqA", name="sqA"),
                                1: ps_s.tile([128, 2, FB], F32, tag="sqB", name="sqB"),
                            }
                            # S matmuls, A/B interleaved so the two 64-row
                            # groups run concurrently in the PE array
                            for i in range(2):
                                ti = q * 2 + i
                                for h_, hp in ((0, 0), (1, 64)):
                                    nc.tensor.matmul(
                                        ps_sq[h_][:, i, :],
                                        KT[hp:hp + 64, gp, ti * 128:(ti + 1) * 128],
                                        QT[hp:hp + 64, gp, j * FB:(j + 1) * FB],
                                        start=True, stop=True,
                                    )
                            for h_ in range(2):
                                ex = p2e.tile([128, 2, FB], BF16,
                                              tag=f"exp{h_}")
                                nc.scalar.activation(
                                    ex[:], ps_sq[h_][:],
                                    mybir.ActivationFunctionType.Exp,
                                    scale=0.125,
                                )
                                nc.vector.tensor_mul(
                                    ex[:], ex[:],
                                    mask_j[:, 2 * q:2 * q + 2, :],
                                )
                                exq[h_] = ex
                            for i in range(2):
                                ti = q * 2 + i
                                for h_, nn in ((0, nA), (1, nB)):
                                    nc.tensor.matmul(
                                        ps_ctx[h_][:],
                                        Vsb[:, ti, nn * HP1:(nn + 1) * HP1],
                                        exq[h_][:, i, :],
                                        start=(ti == 0),
                                        stop=(ti == NT - 1),
                                    )
                        for h_, nn in ((0, nA), (1, nB)):
                            ctx_sb = p2s.tile([HP1, FB], F32, tag=f"ctx{h_}")
                            nc.vector.tensor_copy(ctx_sb[:], ps_ctx[h_][:])
                            # gather this head's sums row onto partition nn
                            nc.sync.dma_start(
                                sums_g[nn:nn + 1, :],
                                ctx_sb[H:H + 1, :],
                            )
                            ctx_keep.append((nn, ctx_sb))
                    # batched normalization for the 4 heads of this j: one
                    # reciprocal on partitions 0-3, then scatter rows to the
                    # 32-aligned partitions the K=1 broadcast matmuls need
                    recip = p2r.tile([128, FB], F32, tag="recip")
                    recip_r = p2r.tile([128, FB], F32R, tag="recipr")
                    nc.vector.reciprocal(recip[0:HPC, :], sums_g[0:HPC, :])
                    nc.vector.tensor_copy(recip_r[0:HPC, :], recip[0:HPC, :])
                    for nn in range(1, HPC):
                        nc.sync.dma_start(
                            recip_r[32 * nn:32 * nn + 1, :],
                            recip_r[nn:nn + 1, :],
                        )
                    for nn, ctx_sb in ctx_keep:
                        ps_bc = ps_s.tile([H, FB], F32, tag="sqA")
                        nc.tensor.matmul(
                            ps_bc[:],
                            ones_r_sb[32 * nn:32 * nn + 1, :],
                            recip_r[32 * nn:32 * nn + 1, :],
                            start=True, stop=True,
                            tile_position=(32 * nn, 0),
                        )
                        out_sb = p2s.tile([H, FB], F32, tag="out")
                        nc.vector.tensor_mul(out_sb[:], ctx_sb[0:H, :], ps_bc[:])
                        nc.sync.dma_start(
                            out_ctx[nn, :, j * FB:(j + 1) * FB], out_sb[:]
                        )

    nc.compile()
    return nc


_compiled = None


def _get_compiled():
    global _compiled
    if _compiled is None:
        _compiled = _program()
    return _compiled


def make_in_maps(from_tensor, to_tensor, attention_mask, wq, bq, wk, bk, wv, bv):
    bf = ml_dtypes.bfloat16
    from_tensor = np.asarray(from_tensor, dtype=np.float32)
    to_tensor = np.asarray(to_tensor, dtype=np.float32)
    attention_mask = np.asarray(attention_mask)
    wq = np.asarray(wq, dtype=np.float32)
    wk = np.asarray(wk, dtype=np.float32)
    wv = np.asarray(wv, dtype=np.float32)
    bq = np.asarray(bq, dtype=np.float32)
    bk = np.asarray(bk, dtype=np.float32)
    bv = np.asarray(bv, dtype=np.float32)

    fromT_b = [np.ascontiguousarray(from_tensor[b].T).astype(bf) for b in range(B)]
    toT_b = [np.ascontiguousarray(to_tensor[b].T).astype(bf) for b in range(B)]
    maskT_b = [attention_mask[b].T.astype(bf) for b in range(B)]
    ones_bf_arr = np.ones((1, 128), dtype=bf)
    ones_r_arr = np.ones((128, H), dtype=np.float32)

    in_maps = []
    for c in range(NCORES):
        b, hb = divmod(c, NCORES // B)
        hs = hb * HPC
        bq_dev = bq[hs:hs + HPC].reshape(NG, 128).T
        bk_dev = bk[hs:hs + HPC].reshape(NG, 128).T
        in_maps.append(
            dict(
                fromT=fromT_b[b],
                toT=toT_b[b],
                maskT=maskT_b[b],
                wq=wq[:, hs:hs + HPC, :].reshape(D, HPC * H).astype(bf),
                wk=wk[:, hs:hs + HPC, :].reshape(D, HPC * H).astype(bf),
                wv=wv[:, hs:hs + HPC, :].reshape(D, HPC * H).astype(bf),
                bqk=np.ascontiguousarray(
                    np.concatenate([bq_dev, bk_dev], axis=1), dtype=np.float32
                ),
                bv=bv[hs:hs + HPC].reshape(1, HPC * H).astype(bf),
                ones_bf=ones_bf_arr,
                ones_r=ones_r_arr,
            )
        )
    return in_maps


def gather_output(results):
    out = np.empty((B, F, N, H), dtype=np.float32)
    for c in range(NCORES):
        b, hb = divmod(c, NCORES // B)
        hs = hb * HPC
        ctx = results[c]["out_ctx"]  # [HPC, H, F]
        out[b, :, hs:hs + HPC, :] = ctx.transpose(2, 0, 1)
    return out


def run_sharded(inputs, **run_kwargs):
    """Run the SPMD kernel; returns (output, BassKernelResults)."""
    nc = _get_compiled()
    in_maps = make_in_maps(**inputs)
    res = run_bass_kernel_spmd(nc, in_maps, list(range(NCORES)), **run_kwargs)
    return gather_output(res.results), res


def kernel(**inputs):
    out, _ = run_sharded(inputs)
    return out


# revision 12
# speedup vs baseline: 1.5073x; 1.3535x over previous
"""Multi-head attention (QKV projection + masked softmax + PV) on 8 TRN2
NeuronCores.

Sharding: data-parallel over batch (B=2 -> 2 groups of 4 cores), tensor
parallel over heads (16 heads -> 4 heads per core). Each core computes full
F x T attention for its 4 heads.

Per-core device algorithm (kept transposed so the softmax reduction lands on
the TensorE contraction dim; all matmuls bf16/fp16, PSUM fp32):
  Q^T[h,f] = wq^T @ from^T        K^T[h,t] = wk^T @ to^T
  V[t,hh]  = to^T^T @ wv   (+ a ones column per head for the softmax sums)
  S^T[t,f] = K^T(stationary, zero-padded to K=128) x Q^T(moving)
  E = exp(S^T/8) (ScalarE, PSUM->SBUF, bf16);  E *= mask^T (bf16)
  ctx^T[h,f] (+ sums[f] via the ones column) = sum_t V x E
  out = ctx^T * (1/sums)   reciprocals batched 4-per-j; broadcast down the
                           h-partitions via a selector-row ones matmul
Every matmul keeps tile_size (128,128) -- K=1 products are zero-padded to
K=128 via selector rows -- so the PE array never drains for a mode switch,
and PV is emitted one quad behind S so the PE stream never stalls long
enough for the HAM clock gate to re-throttle.

Host does the cheap layout work: pre-transposes from/to/mask (bf16), slices
weights per head group, transposes the [4,64,2048] per-core results back into
[B,F,N,H].
"""

import os
import sys

for _p in ("/opt/trn_rl_repo",):
    if os.path.isdir(_p) and _p not in sys.path:
        sys.path.insert(0, _p)

import numpy as np
import ml_dtypes

import concourse.tile as tile
from concourse import bacc, mybir
from concourse.bass_utils import run_bass_kernel_spmd

B, F, T, D, N, H = 2, 2048, 2048, 1024, 16, 64
NCORES = 8
HPC = N // (NCORES // B)  # heads per core = 4
NG = HPC // 2             # 128-partition head groups (2 heads each) = 2
FB = 512                  # f-block (psum bank width in fp32)
NJ = F // FB              # 4
NT = T // 128             # 16 t-tiles
NK = D // 128             # 8 contraction tiles
HP1 = H + 1               # head V columns incl. the ones column

F32 = mybir.dt.float32
F16 = mybir.dt.float16
BF16 = mybir.dt.bfloat16


def _phase1(nc, tc, persist, tensors):
    (fromT, toT, wq, wk, wv) = tensors["dram"]
    (QT, KTe, KTo, Vsb, bias_sb, bv_sb, vones_sb) = tensors["sbuf"]
    with (
        tc.tile_pool(name="p1", bufs=1) as p1,
        tc.tile_pool(name="ps1", bufs=3, space="PSUM") as ps1,
    ):
        toT_sb = p1.tile([128, NK, T], BF16)
        fromT_sb = p1.tile([128, NK, F], BF16)
        wq_sb = p1.tile([128, NK, HPC * H], BF16)
        wk_sb = p1.tile([128, NK, HPC * H], BF16)
        wv_sb = p1.tile([128, NK, HPC * H], BF16)
        nc.sync.dma_start(wq_sb[:], wq[:].rearrange("(k p) m -> p k m", p=128))
        nc.sync.dma_start(wk_sb[:], wk[:].rearrange("(k p) m -> p k m", p=128))
        nc.sync.dma_start(wv_sb[:], wv[:].rearrange("(k p) m -> p k m", p=128))
        for k in range(NK):
            nc.sync.dma_start(toT_sb[:, k, :], toT[k * 128:(k + 1) * 128, :])
            nc.sync.dma_start(fromT_sb[:, k, :], fromT[k * 128:(k + 1) * 128, :])

        for w_sb, src_sb, bcol in (
            (wk_sb, toT_sb, NG),
            (wq_sb, fromT_sb, 0),
        ):
            for g in range(NG):
                for j in range(NJ):
                    ps_qk = ps1.tile([128, FB], F32, tag="qk")
                    for k in range(NK):
                        nc.tensor.matmul(
                            ps_qk[:],
                            w_sb[:, k, g * 128:(g + 1) * 128],
                            src_sb[:, k, j * FB:(j + 1) * FB],
                            start=(k == 0),
                            stop=(k == NK - 1),
                        )
                    if bcol == 0:  # Q^T: packed head pair
                        nc.vector.tensor_scalar_add(
                            QT[:, g, j * FB:(j + 1) * FB],
                            ps_qk[:],
                            bias_sb[:, g:g + 1],
                        )
                    else:  # K^T: split by head parity, zero-padded halves
                        nc.vector.tensor_scalar_add(
                            KTe[0:64, g, j * FB:(j + 1) * FB],
                            ps_qk[0:64, :],
                            bias_sb[0:64, bcol + g:bcol + g + 1],
                        )
                        nc.vector.tensor_scalar_add(
                            KTo[64:128, g, j * FB:(j + 1) * FB],
                            ps_qk[64:128, :],
                            bias_sb[64:128, bcol + g:bcol + g + 1],
                        )

        for ti in range(NT):
            ps_v = ps1.tile([128, HPC * H], F32, tag="v")
            for k in range(NK):
                nc.tensor.matmul(
                    ps_v[:],
                    toT_sb[:, k, ti * 128:(ti + 1) * 128],
                    wv_sb[:, k, :],
                    start=(k == 0),
                    stop=False,
                )
            nc.tensor.matmul(ps_v[:], vones_sb[:], bv_sb[:], start=False, stop=True)
            for nl in range(HPC):
                nc.vector.tensor_copy(
                    Vsb[:, ti, nl * HP1:nl * HP1 + H],
                    ps_v[:, nl * H:(nl + 1) * H],
                )


def _program():
    nc = bacc.Bacc(None, target_bir_lowering=False)
    fromT = nc.declare_dram_parameter("fromT", [D, F], BF16, isOutput=False)
    toT = nc.declare_dram_parameter("toT", [D, T], BF16, isOutput=False)
    maskT = nc.declare_dram_parameter("maskT", [T, F], BF16, isOutput=False)
    wq = nc.declare_dram_parameter("wq", [D, HPC * H], BF16, isOutput=False)
    wk = nc.declare_dram_parameter("wk", [D, HPC * H], BF16, isOutput=False)
    wv = nc.declare_dram_parameter("wv", [D, HPC * H], BF16, isOutput=False)
    bqk = nc.declare_dram_parameter("bqk", [128, 2 * NG], F32, isOutput=False)
    # bv padded to K=128 (row 0 = bv, rest zero) for a mode-switch-free matmul
    bv_pad = nc.declare_dram_parameter("bv_pad", [128, HPC * H], BF16, isOutput=False)
    # all-ones row 0 (rest zero): stationary operand of the bv matmul
    vones = nc.declare_dram_parameter("vones", [128, 128], BF16, isOutput=False)
    # selector blocks: ones_bc[k, nn, m] = (k == nn), broadcast matmul lhsT
    ones_bc = nc.declare_dram_parameter("ones_bc", [128, HPC, 128], F16, isOutput=False)
    out_ctx = nc.declare_dram_parameter("out_ctx", [HPC, H, F], F32, isOutput=True)

    with tile.TileContext(nc) as tc:
        with tc.tile_pool(name="persist", bufs=1) as persist:
            QT = persist.tile([128, NG, F], BF16)        # [h-in-group, g, f]
            # K^T per head parity, dead half zeroed so S can contract K=128
            KTe = persist.tile([128, NG, T], BF16)       # heads 2g   in rows 0-63
            KTo = persist.tile([128, NG, T], BF16)       # heads 2g+1 in rows 64-127
            Vsb = persist.tile([128, NT, HPC * HP1], BF16)
            bias_sb = persist.tile([128, 2 * NG], F32)
            bv_sb = persist.tile([128, HPC * H], BF16)
            vones_sb = persist.tile([128, 128], BF16)
            ones_bc_sb = persist.tile([128, HPC, 128], F16)
            nc.sync.dma_start(bias_sb[:], bqk[:])
            nc.sync.dma_start(bv_sb[:], bv_pad[:])
            nc.sync.dma_start(vones_sb[:], vones[:])
            nc.sync.dma_start(ones_bc_sb[:], ones_bc[:])
            nc.vector.memset(KTe[64:128, :, :], 0.0)
            nc.vector.memset(KTo[0:64, :, :], 0.0)
            for nl in range(HPC):
                nc.vector.memset(Vsb[:, :, nl * HP1 + H], 1.0)

            with tc.tile_pool(name="p2", bufs=2) as p2:
                # prefetch the first mask block before phase-1 floods the DMAs
                maskT_re = maskT[:].rearrange("(a p) f -> p a f", p=128)
                masks = {}
                masks[0] = p2.tile([128, NT, FB], BF16, tag="mask", name="mask")
                nc.sync.dma_start(masks[0][:], maskT_re[:, :, 0:FB])

                _phase1(nc, tc, persist, dict(
                    dram=(fromT, toT, wq, wk, wv),
                    sbuf=(QT, KTe, KTo, Vsb, bias_sb, bv_sb, vones_sb),
                ))

                # ---- phase 2: attention ----
                with (
                    tc.tile_pool(name="p2e", bufs=3) as p2e,
                    tc.tile_pool(name="p2s", bufs=3) as p2s,
                    tc.tile_pool(name="p2r", bufs=2) as p2r,
                    tc.tile_pool(name="ps_s", bufs=3, space="PSUM") as ps_s,
                    tc.tile_pool(name="ps_c", bufs=1, space="PSUM") as ps_c,
                ):
                    for j in range(NJ):
                        mask_j = masks.pop(j)
                        if j + 1 < NJ:
                            masks[j + 1] = p2.tile([128, NT, FB], BF16,
                                                   tag="mask", name="mask")
                            nc.sync.dma_start(
                                masks[j + 1][:],
                                maskT_re[:, :, (j + 1) * FB:(j + 2) * FB],
                            )
                        sums_g = p2r.tile([128, FB], F32, tag="sums")
                        ctx_keep = []
                        for gp in range(NG):
                            ps_ctx = {
                                0: ps_c.tile([HP1, FB], F32, tag="ctxA", name="ctxA"),
                                1: ps_c.tile([HP1, FB], F32, tag="ctxB", name="ctxB"),
                            }
                            pend = None  # PV runs one quad behind S/exp
                            for q in range(NT // 2):
                                ps_sq = {
                                    0: ps_s.tile([128, 2, FB], F32, tag="sq", name="sqA"),
                                    1: ps_s.tile([128, 2, FB], F32, tag="sq", name="sqB"),
                                }
                                for i in range(2):
                                    ti = q * 2 + i
                                    for h_, KT_ in ((0, KTe), (1, KTo)):
                                        nc.tensor.matmul(
                                            ps_sq[h_][:, i, :],
                                            KT_[:, gp, ti * 128:(ti + 1) * 128],
                                            QT[:, gp, j * FB:(j + 1) * FB],
                                            start=True, stop=True,
                                        )
                                exq = {}
                                for h_ in range(2):
                                    ex = p2e.tile([128, 2, FB], BF16,
                                                  tag=f"exp{h_}", name=f"exp{h_}")
                                    nc.scalar.activation(
                                        ex[:], ps_sq[h_][:],
                                        mybir.ActivationFunctionType.Exp,
                                        scale=0.125,
                                    )
                                    nc.vector.tensor_mul(
                                        ex[:], ex[:], mask_j[:, 2 * q:2 * q + 2, :]
                                    )
                                    exq[h_] = ex
                                if pend is not None:
                                    pq, pexq = pend
                                    for i in range(2):
                                        ti = pq * 2 + i
                                        for h_ in range(2):
                                            nn = 2 * gp + h_
                                            nc.tensor.matmul(
                                                ps_ctx[h_][:],
                                                Vsb[:, ti, nn * HP1:(nn + 1) * HP1],
                                                pexq[h_][:, i, :],
                                                start=(ti == 0),
                                                stop=False,
                                            )
                                pend = (q, exq)
                            pq, pexq = pend
                            for i in range(2):
                                ti = pq * 2 + i
                                for h_ in range(2):
                                    nn = 2 * gp + h_
                                    nc.tensor.matmul(
                                        ps_ctx[h_][:],
                                        Vsb[:, ti, nn * HP1:(nn + 1) * HP1],
                                        pexq[h_][:, i, :],
                                        start=False,
                                        stop=(ti == NT - 1),
                                    )
                            for h_ in range(2):
                                nn = 2 * gp + h_
                                ctx_sb = p2s.tile([HP1, FB], F32,
                                                  tag=f"ctx{h_}", name=f"ctx{h_}")
                                nc.vector.tensor_copy(ctx_sb[:], ps_ctx[h_][:])
                                # gather this head's sums row onto partition nn
                                nc.sync.dma_start(
                                    sums_g[nn:nn + 1, :], ctx_sb[H:H + 1, :]
                                )
                                ctx_keep.append((nn, ctx_sb))
                        # batched normalization for the 4 heads of this j
                        recip = p2r.tile([128, FB], F32, tag="recip")
                        recip_h = p2r.tile([128, FB], F16, tag="reciph")
                        nc.vector.reciprocal(recip[0:HPC, :], sums_g[0:HPC, :])
                        nc.vector.memset(recip_h[:], 0.0)
                        nc.vector.tensor_copy(recip_h[0:HPC, :], recip[0:HPC, :])
                        for nn, ctx_sb in ctx_keep:
                            ps_bc = ps_s.tile([128, FB], F32, tag="sq", name="ps_bc")
                            nc.tensor.matmul(
                                ps_bc[:], ones_bc_sb[:, nn, :], recip_h[:],
                                start=True, stop=True,
                            )
                            out_sb = p2s.tile([H, FB], F32, tag="out")
                            nc.vector.tensor_mul(
                                out_sb[:], ctx_sb[0:H, :], ps_bc[0:H, :]
                            )
                            nc.sync.dma_start(
                                out_ctx[nn, :, j * FB:(j + 1) * FB], out_sb[:]
                            )

    nc.compile()
    return nc


_compiled = None


def _get_compiled():
    global _compiled
    if _compiled is None:
        _compiled = _program()
    return _compiled


def make_in_maps(from_tensor, to_tensor, attention_mask, wq, bq, wk, bk, wv, bv):
    bf = ml_dtypes.bfloat16
    from_tensor = np.asarray(from_tensor, dtype=np.float32)
    to_tensor = np.asarray(to_tensor, dtype=np.float32)
    attention_mask = np.asarray(attention_mask)
    wq = np.asarray(wq, dtype=np.float32)
    wk = np.asarray(wk, dtype=np.float32)
    wv = np.asarray(wv, dtype=np.float32)
    bq = np.asarray(bq, dtype=np.float32)
    bk = np.asarray(bk, dtype=np.float32)
    bv = np.asarray(bv, dtype=np.float32)

    fromT_b = [np.ascontiguousarray(from_tensor[b].T).astype(bf) for b in range(B)]
    toT_b = [np.ascontiguousarray(to_tensor[b].T).astype(bf) for b in range(B)]
    maskT_b = [attention_mask[b].T.astype(bf) for b in range(B)]
    vones_arr = np.zeros((128, 128), dtype=bf)
    vones_arr[0, :] = 1.0
    ones_bc_arr = np.zeros((128, HPC, 128), dtype=np.float16)
    for nn in range(HPC):
        ones_bc_arr[nn, nn, :] = 1.0

    in_maps = []
    for c in range(NCORES):
        b, hb = divmod(c, NCORES // B)
        hs = hb * HPC
        bq_dev = bq[hs:hs + HPC].reshape(NG, 128).T
        bk_dev = bk[hs:hs + HPC].reshape(NG, 128).T
        bv_pad = np.zeros((128, HPC * H), dtype=bf)
        bv_pad[0, :] = bv[hs:hs + HPC].reshape(HPC * H)
        in_maps.append(
            dict(
                fromT=fromT_b[b],
                toT=toT_b[b],
                maskT=maskT_b[b],
                wq=wq[:, hs:hs + HPC, :].reshape(D, HPC * H).astype(bf),
                wk=wk[:, hs:hs + HPC, :].reshape(D, HPC * H).astype(bf),
                wv=wv[:, hs:hs + HPC, :].reshape(D, HPC * H).astype(bf),
                bqk=np.ascontiguousarray(
                    np.concatenate([bq_dev, bk_dev], axis=1), dtype=np.float32
                ),
                bv_pad=bv_pad,
                vones=vones_arr,
                ones_bc=ones_bc_arr,
            )
        )
    return in_maps


def gather_output(results):
    out = np.empty((B, F, N, H), dtype=np.float32)
    for c in range(NCORES):
        b, hb = divmod(c, NCORES // B)
        hs = hb * HPC
        ctx = results[c]["out_ctx"]  # [HPC, H, F]
        out[b, :, hs:hs + HPC, :] = ctx.transpose(2, 0, 1)
    return out


def run_sharded(inputs, **run_kwargs):
    """Run the SPMD kernel; returns (output, BassKernelResults)."""
    nc = _get_compiled()
    in_maps = make_in_maps(**inputs)
    res = run_bass_kernel_spmd(nc, in_maps, list(range(NCORES)), **run_kwargs)
    return gather_output(res.results), res


def kernel(**inputs):
    out, _ = run_sharded(inputs)
    return out


# revision 13
# speedup vs baseline: 1.6705x; 1.1083x over previous
"""Multi-head attention (QKV projection + masked softmax + PV) on 8 TRN2
NeuronCores.

Sharding: data-parallel over batch (B=2 -> 2 groups of 4 cores), tensor
parallel over heads (16 heads -> 4 heads per core). Each core computes full
F x T attention for its 4 heads.

Per-core device algorithm (kept transposed so the softmax reduction lands on
the TensorE contraction dim; all matmuls bf16/fp16, PSUM fp32):
  Q^T[h,f] = wq^T @ from^T        K^T[h,t] = wk^T @ to^T
  V[t,hh]  = to^T^T @ wv   (+ a ones column per head for the softmax sums)
  S^T[t,f] = K^T(stationary, zero-padded to K=128) x Q^T(moving)
  E = exp(S^T/8) (ScalarE, PSUM->SBUF, bf16);  E *= mask^T (bf16)
  ctx^T[h,f] (+ sums[f] via the ones column) = sum_t V x E
  out = ctx^T * (1/sums)   reciprocals batched 4-per-j; broadcast down the
                           h-partitions via a selector-row ones matmul
Every matmul keeps tile_size (128,128) -- K=1 products are zero-padded to
K=128 via selector rows -- so the PE array never drains for a mode switch,
and PV is emitted one quad behind S so the PE stream never stalls long
enough for the HAM clock gate to re-throttle.

Host does the cheap layout work: pre-transposes from/to/mask (bf16), slices
weights per head group, transposes the [4,64,2048] per-core results back into
[B,F,N,H].
"""

import os
import sys

for _p in ("/opt/trn_rl_repo",):
    if os.path.isdir(_p) and _p not in sys.path:
        sys.path.insert(0, _p)

import numpy as np
import ml_dtypes

import concourse.tile as tile
from concourse import bacc, mybir
from concourse.bass_utils import run_bass_kernel_spmd

B, F, T, D, N, H = 2, 2048, 2048, 1024, 16, 64
NCORES = 8
HPC = N // (NCORES // B)  # heads per core = 4
NG = HPC // 2             # 128-partition head groups (2 heads each) = 2
FB = 512                  # f-block (psum bank width in fp32)
NJ = F // FB              # 4
NT = T // 128             # 16 t-tiles
NK = D // 128             # 8 contraction tiles
HP1 = H + 1               # head V columns incl. the ones column

F32 = mybir.dt.float32
F16 = mybir.dt.float16
BF16 = mybir.dt.bfloat16


def _phase1(nc, tc, persist, tensors):
    (fromT, toT, wq, wk, wv) = tensors["dram"]
    (QT, KTe, KTo, Vsb, bias_sb, bv_sb, vones_sb) = tensors["sbuf"]
    with (
        tc.tile_pool(name="p1", bufs=1) as p1,
        tc.tile_pool(name="ps1", bufs=3, space="PSUM") as ps1,
    ):
        toT_sb = p1.tile([128, NK, T], BF16)
        fromT_sb = p1.tile([128, NK, F], BF16)
        wq_sb = p1.tile([128, NK, HPC * H], BF16)
        wk_sb = p1.tile([128, NK, HPC * H], BF16)
        wv_sb = p1.tile([128, NK, HPC * H], BF16)
        nc.sync.dma_start(wq_sb[:], wq[:].rearrange("(k p) m -> p k m", p=128))
        nc.sync.dma_start(wk_sb[:], wk[:].rearrange("(k p) m -> p k m", p=128))
        nc.sync.dma_start(wv_sb[:], wv[:].rearrange("(k p) m -> p k m", p=128))
        for j in range(NJ):
            for k in range(NK):
                nc.sync.dma_start(
                    toT_sb[:, k, j * FB:(j + 1) * FB],
                    toT[k * 128:(k + 1) * 128, j * FB:(j + 1) * FB],
                )
        for j in range(NJ):
            for k in range(NK):
                nc.sync.dma_start(
                    fromT_sb[:, k, j * FB:(j + 1) * FB],
                    fromT[k * 128:(k + 1) * 128, j * FB:(j + 1) * FB],
                )

        for w_sb, src_sb, bcol in (
            (wk_sb, toT_sb, NG),
            (wq_sb, fromT_sb, 0),
        ):
            for g in range(NG):
                for j in range(NJ):
                    ps_qk = ps1.tile([128, FB], F32, tag="qk")
                    for k in range(NK):
                        nc.tensor.matmul(
                            ps_qk[:],
                            w_sb[:, k, g * 128:(g + 1) * 128],
                            src_sb[:, k, j * FB:(j + 1) * FB],
                            start=(k == 0),
                            stop=(k == NK - 1),
                        )
                    if bcol == 0:  # Q^T: packed head pair
                        nc.vector.tensor_scalar_add(
                            QT[:, g, j * FB:(j + 1) * FB],
                            ps_qk[:],
                            bias_sb[:, g:g + 1],
                        )
                    else:  # K^T: split by head parity, zero-padded halves
                        nc.vector.tensor_scalar_add(
                            KTe[0:64, g, j * FB:(j + 1) * FB],
                            ps_qk[0:64, :],
                            bias_sb[0:64, bcol + g:bcol + g + 1],
                        )
                        nc.vector.tensor_scalar_add(
                            KTo[64:128, g, j * FB:(j + 1) * FB],
                            ps_qk[64:128, :],
                            bias_sb[64:128, bcol + g:bcol + g + 1],
                        )

        for ti in range(NT):
            ps_v = ps1.tile([128, HPC * H], F32, tag="v")
            for k in range(NK):
                nc.tensor.matmul(
                    ps_v[:],
                    toT_sb[:, k, ti * 128:(ti + 1) * 128],
                    wv_sb[:, k, :],
                    start=(k == 0),
                    stop=False,
                )
            nc.tensor.matmul(ps_v[:], vones_sb[:], bv_sb[:], start=False, stop=True)
            for nl in range(HPC):
                nc.vector.tensor_copy(
                    Vsb[:, ti, nl * HP1:nl * HP1 + H],
                    ps_v[:, nl * H:(nl + 1) * H],
                )


def _program():
    nc = bacc.Bacc(None, target_bir_lowering=False)
    fromT = nc.declare_dram_parameter("fromT", [D, F], BF16, isOutput=False)
    toT = nc.declare_dram_parameter("toT", [D, T], BF16, isOutput=False)
    maskT = nc.declare_dram_parameter("maskT", [T, F], BF16, isOutput=False)
    wq = nc.declare_dram_parameter("wq", [D, HPC * H], BF16, isOutput=False)
    wk = nc.declare_dram_parameter("wk", [D, HPC * H], BF16, isOutput=False)
    wv = nc.declare_dram_parameter("wv", [D, HPC * H], BF16, isOutput=False)
    bqk = nc.declare_dram_parameter("bqk", [128, 2 * NG], F32, isOutput=False)
    # bv padded to K=128 (row 0 = bv, rest zero) for a mode-switch-free matmul
    bv_pad = nc.declare_dram_parameter("bv_pad", [128, HPC * H], BF16, isOutput=False)
    # all-ones row 0 (rest zero): stationary operand of the bv matmul
    vones = nc.declare_dram_parameter("vones", [128, 128], BF16, isOutput=False)
    # selector blocks: ones_bc[k, nn, m] = (k == nn), broadcast matmul lhsT
    ones_bc = nc.declare_dram_parameter("ones_bc", [128, HPC, 128], F16, isOutput=False)
    out_ctx = nc.declare_dram_parameter("out_ctx", [HPC, H, F], F32, isOutput=True)

    with tile.TileContext(nc) as tc:
        with tc.tile_pool(name="persist", bufs=1) as persist:
            QT = persist.tile([128, NG, F], BF16)        # [h-in-group, g, f]
            # K^T per head parity, dead half zeroed so S can contract K=128
            KTe = persist.tile([128, NG, T], BF16)       # heads 2g   in rows 0-63
            KTo = persist.tile([128, NG, T], BF16)       # heads 2g+1 in rows 64-127
            Vsb = persist.tile([128, NT, HPC * HP1], BF16)
            bias_sb = persist.tile([128, 2 * NG], F32)
            bv_sb = persist.tile([128, HPC * H], BF16)
            vones_sb = persist.tile([128, 128], BF16)
            ones_bc_sb = persist.tile([128, HPC, 128], F16)
            nc.sync.dma_start(bias_sb[:], bqk[:])
            nc.sync.dma_start(bv_sb[:], bv_pad[:])
            nc.sync.dma_start(vones_sb[:], vones[:])
            nc.sync.dma_start(ones_bc_sb[:], ones_bc[:])
            act_warm = persist.tile([1, 1], F32)
            nc.scalar.activation(act_warm[:], bias_sb[0:1, 0:1],
                                 mybir.ActivationFunctionType.Exp)
            nc.vector.memset(KTe[64:128, :, :], 0.0)
            nc.vector.memset(KTo[0:64, :, :], 0.0)
            for nl in range(HPC):
                nc.vector.memset(Vsb[:, :, nl * HP1 + H], 1.0)

            with tc.tile_pool(name="p2", bufs=2) as p2:
                # prefetch the first mask block before phase-1 floods the DMAs
                maskT_re = maskT[:].rearrange("(a p) f -> p a f", p=128)
                masks = {}
                masks[0] = p2.tile([128, NT, FB], BF16, tag="mask", name="mask")
                nc.sync.dma_start(masks[0][:], maskT_re[:, :, 0:FB])

                _phase1(nc, tc, persist, dict(
                    dram=(fromT, toT, wq, wk, wv),
                    sbuf=(QT, KTe, KTo, Vsb, bias_sb, bv_sb, vones_sb),
                ))

                # ---- phase 2: attention ----
                with (
                    tc.tile_pool(name="p2e", bufs=4) as p2e,
                    tc.tile_pool(name="p2s", bufs=3) as p2s,
                    tc.tile_pool(name="p2r", bufs=2) as p2r,
                    tc.tile_pool(name="ps_s", bufs=2, space="PSUM") as ps_s,
                    tc.tile_pool(name="ps_c", bufs=1, space="PSUM") as ps_c,
                    tc.tile_pool(name="ps_b", bufs=2, space="PSUM") as ps_b,
                ):
                    for j in range(NJ):
                        mask_j = masks.pop(j)
                        if j + 1 < NJ:
                            masks[j + 1] = p2.tile([128, NT, FB], BF16,
                                                   tag="mask", name="mask")
                            nc.sync.dma_start(
                                masks[j + 1][:],
                                maskT_re[:, :, (j + 1) * FB:(j + 2) * FB],
                            )
                        sums_g = p2r.tile([128, FB], F32, tag="sums")
                        ctx_keep = []
                        for gp in range(NG):
                            ps_ctx = {
                                0: ps_c.tile([HP1, FB], F32, tag="ctxA", name="ctxA"),
                                1: ps_c.tile([HP1, FB], F32, tag="ctxB", name="ctxB"),
                            }
                            pend = None  # PV runs one quad behind S/exp
                            for q in range(NT // 2):
                                ps_sq = {
                                    0: ps_s.tile([128, 2, FB], F32, tag="sq", name="sqA"),
                                    1: ps_s.tile([128, 2, FB], F32, tag="sq", name="sqB"),
                                }
                                for i in range(2):
                                    ti = q * 2 + i
                                    for h_, KT_ in ((0, KTe), (1, KTo)):
                                        nc.tensor.matmul(
                                            ps_sq[h_][:, i, :],
                                            KT_[:, gp, ti * 128:(ti + 1) * 128],
                                            QT[:, gp, j * FB:(j + 1) * FB],
                                            start=True, stop=True,
                                        )
                                exq = {}
                                for h_ in range(2):
                                    ex = p2e.tile([128, 2, FB], BF16,
                                                  tag=f"exp{h_}", name=f"exp{h_}")
                                    nc.scalar.activation(
                                        ex[:], ps_sq[h_][:],
                                        mybir.ActivationFunctionType.Exp,
                                        scale=0.125,
                                    )
                                    nc.vector.tensor_mul(
                                        ex[:], ex[:], mask_j[:, 2 * q:2 * q + 2, :]
                                    )
                                    exq[h_] = ex
                                if pend is not None:
                                    pq, pexq = pend
                                    for i in range(2):
                                        ti = pq * 2 + i
                                        for h_ in range(2):
                                            nn = 2 * gp + h_
                                            nc.tensor.matmul(
                                                ps_ctx[h_][:],
                                                Vsb[:, ti, nn * HP1:(nn + 1) * HP1],
                                                pexq[h_][:, i, :],
                                                start=(ti == 0),
                                                stop=False,
                                            )
                                pend = (q, exq)
                            pq, pexq = pend
                            for i in range(2):
                                ti = pq * 2 + i
                                for h_ in range(2):
                                    nn = 2 * gp + h_
                                    nc.tensor.matmul(
                                        ps_ctx[h_][:],
                                        Vsb[:, ti, nn * HP1:(nn + 1) * HP1],
                                        pexq[h_][:, i, :],
                                        start=False,
                                        stop=(ti == NT - 1),
                                    )
                            for h_ in range(2):
                                nn = 2 * gp + h_
                                ctx_sb = p2s.tile([HP1, FB], F32,
                                                  tag=f"ctx{h_}", name=f"ctx{h_}")
                                nc.vector.tensor_copy(ctx_sb[:], ps_ctx[h_][:])
                                # gather this head's sums row onto partition nn
                                nc.sync.dma_start(
                                    sums_g[nn:nn + 1, :], ctx_sb[H:H + 1, :]
                                )
                                ctx_keep.append((nn, ctx_sb))
                        # batched normalization for the 4 heads of this j
                        recip = p2r.tile([128, FB], F32, tag="recip")
                        recip_h = p2r.tile([128, FB], F16, tag="reciph")
                        nc.vector.reciprocal(recip[0:HPC, :], sums_g[0:HPC, :])
                        nc.vector.memset(recip_h[:], 0.0)
                        nc.vector.tensor_copy(recip_h[0:HPC, :], recip[0:HPC, :])
                        for nn, ctx_sb in ctx_keep:
                            ps_bc = ps_b.tile([128, FB], F32, tag="bc", name="ps_bc")
                            nc.tensor.matmul(
                                ps_bc[:], ones_bc_sb[:, nn, :], recip_h[:],
                                start=True, stop=True,
                            )
                            out_sb = p2s.tile([H, FB], F32, tag="out")
                            nc.vector.tensor_mul(
                                out_sb[:], ctx_sb[0:H, :], ps_bc[0:H, :]
                            )
                            nc.sync.dma_start(
                                out_ctx[nn, :, j * FB:(j + 1) * FB], out_sb[:]
                            )

    nc.compile()
    return nc


_compiled = None


def _get_compiled():
    global _compiled
    if _compiled is None:
        _compiled = _program()
    return _compiled


def make_in_maps(from_tensor, to_tensor, attention_mask, wq, bq, wk, bk, wv, bv):
    bf = ml_dtypes.bfloat16
    from_tensor = np.asarray(from_tensor, dtype=np.float32)
    to_tensor = np.asarray(to_tensor, dtype=np.float32)
    attention_mask = np.asarray(attention_mask)
    wq = np.asarray(wq, dtype=np.float32)
    wk = np.asarray(wk, dtype=np.float32)
    wv = np.asarray(wv, dtype=np.float32)
    bq = np.asarray(bq, dtype=np.float32)
    bk = np.asarray(bk, dtype=np.float32)
    bv = np.asarray(bv, dtype=np.float32)

    fromT_b = [np.ascontiguousarray(from_tensor[b].T).astype(bf) for b in range(B)]
    toT_b = [np.ascontiguousarray(to_tensor[b].T).astype(bf) for b in range(B)]
    maskT_b = [attention_mask[b].T.astype(bf) for b in range(B)]
    vones_arr = np.zeros((128, 128), dtype=bf)
    vones_arr[0, :] = 1.0
    ones_bc_arr = np.zeros((128, HPC, 128), dtype=np.float16)
    for nn in range(HPC):
        ones_bc_arr[nn, nn, :] = 1.0

    in_maps = []
    for c in range(NCORES):
        b, hb = divmod(c, NCORES // B)
        hs = hb * HPC
        bq_dev = bq[hs:hs + HPC].reshape(NG, 128).T
        bk_dev = bk[hs:hs + HPC].reshape(NG, 128).T
        bv_pad = np.zeros((128, HPC * H), dtype=bf)
        bv_pad[0, :] = bv[hs:hs + HPC].reshape(HPC * H)
        in_maps.append(
            dict(
                fromT=fromT_b[b],
                toT=toT_b[b],
                maskT=maskT_b[b],
                wq=wq[:, hs:hs + HPC, :].reshape(D, HPC * H).astype(bf),
                wk=wk[:, hs:hs + HPC, :].reshape(D, HPC * H).astype(bf),
                wv=wv[:, hs:hs + HPC, :].reshape(D, HPC * H).astype(bf),
                bqk=np.ascontiguousarray(
                    np.concatenate([bq_dev, bk_dev], axis=1), dtype=np.float32
                ),
                bv_pad=bv_pad,
                vones=vones_arr,
                ones_bc=ones_bc_arr,
            )
        )
    return in_maps


def gather_output(results):
    out = np.empty((B, F, N, H), dtype=np.float32)
    for c in range(NCORES):
        b, hb = divmod(c, NCORES // B)
        hs = hb * HPC
        ctx = results[c]["out_ctx"]  # [HPC, H, F]
        out[b, :, hs:hs + HPC, :] = ctx.transpose(2, 0, 1)
    return out


def run_sharded(inputs, **run_kwargs):
    """Run the SPMD kernel; returns (output, BassKernelResults)."""
    nc = _get_compiled()
    in_maps = make_in_maps(**inputs)
    res = run_bass_kernel_spmd(nc, in_maps, list(range(NCORES)), **run_kwargs)
    return gather_output(res.results), res


def kernel(**inputs):
    out, _ = run_sharded(inputs)
    return out
